# revision 18
# baseline (speedup 1.0000x reference)
"""AttnBlock (GroupNorm + 4-head d=128 self-attention + residual).

Full input x: [8, 512, 2048] fp32. Data-parallel over batch: core b computes
batch b entirely on-chip (no collectives).

Per-core math (C=512, L=2048, G=4 groups, NH=4 heads, HD=128):
  h  = groupnorm(x)                    (group == one 128-partition tile)
  q  = wq @ h + bq   [d, l] layout     (PE-transposed weights)
  k  = wk @ h + bk   [d, l]
  vT = h^T @ wv^T + bv  [l, d] layout  (produced transposed; no V transposes)
  sT[k,q] = k_chunk^T q  -> exp (no max-sub; logits ~ N(0,1))
  den = ones^T exp (cross-partition sum, broadcast to 128 partitions)
  avT[d,q] = sum_kt vT_chunk^T exp_chunk ; attn = avT * (1/den)
  out = wo @ attn + bo + x

Matmuls run as float32r (full-rate fp32 mode). fp32r is a distinct lossy bit
layout (~2^-12 relative); every fp32r operand is produced by a compute engine
writing dtype float32r (conversions folded into PSUM->SBUF moves that exist
anyway; those moves run on the scalar engine to keep the vector engine off
the PSUM-drain critical path).

Scheduling: weight loads + PE transposes are emitted first so the tensor
engine has work while groupnorm stats stream in; the attention inner loop is
software-pipelined per k-tile (QK+exp of tile i+1 ahead of den/av of tile i);
the attention loop is q-chunk-outer and the output projection for each
l-chunk uses a dedicated PSUM slot so it overlaps the next q-chunk's
attention.

PSUM budget (8 banks): s4 3x1 + den 2x1 + av 2x1 + op 1x1.
"""

import os
import numpy as np

import concourse.bass as bass
import concourse.tile as tile
from concourse import bacc, mybir
from concourse.bass_utils import run_bass_kernel_spmd
from concourse.masks import make_identity

F32 = mybir.dt.float32
F32R = mybir.dt.float32r

B, C, L = 8, 512, 2048
G = 4            # groupnorm groups; group size 128 == one partition tile
NH, HD = 4, 128  # heads, head dim
CT = C // 128    # 4 channel tiles
LC = L // 512    # 4 l-chunks of 512
LT = L // 128    # 16 l-tiles of 128
EPS = 1e-6
SM_SCALE = float(HD) ** -0.5

AFT = mybir.ActivationFunctionType
ALU = mybir.AluOpType


def build_attn_block(nc):
    x_d = nc.dram_tensor("x", [C, L], F32, kind="ExternalInput").ap()
    gs_d = nc.dram_tensor("gn_scale", [C], F32, kind="ExternalInput").ap()
    gb_d = nc.dram_tensor("gn_bias", [C], F32, kind="ExternalInput").ap()
    w_d = {}
    b_d = {}
    for nm in ("q", "k", "v", "o"):
        w_d[nm] = nc.dram_tensor(f"w{nm}", [C, C], F32, kind="ExternalInput").ap()
        b_d[nm] = nc.dram_tensor(f"b{nm}", [C], F32, kind="ExternalInput").ap()
    out_d = nc.dram_tensor("out", [C, L], F32, kind="ExternalOutput").ap()

    with tile.TileContext(nc) as tc:
        with (
            tc.tile_pool(name="const", bufs=1) as const,
            tc.tile_pool(name="wt", bufs=1) as wt,
            tc.tile_pool(name="big", bufs=1) as big,
            tc.tile_pool(name="small", bufs=4) as small,
            tc.tile_pool(name="epool", bufs=4) as epool,
            tc.tile_pool(name="cpool", bufs=2) as cpool,
            tc.tile_pool(name="psum", bufs=2, space="PSUM") as psum,
        ):
            # ---- constants ----
            identity = const.tile([128, 128], F32)
            make_identity(nc, identity)
            ones = const.tile([128, 128], F32)
            nc.vector.memset(ones, 1.0)
            ones_r = const.tile([128, 128], F32R)
            nc.vector.tensor_copy(ones_r, ones)
            eps_t = const.tile([128, 1], F32)
            nc.vector.memset(eps_t, EPS)

            def load_cvec(name, ap_1d):
                t = const.tile([128, CT], F32, name=name)
                nc.sync.dma_start(out=t, in_=ap_1d.rearrange("(t p) -> p t", p=128))
                return t

            bq_sb = load_cvec("bq_sb", b_d["q"])
            bk_sb = load_cvec("bk_sb", b_d["k"])
            bo_sb = load_cvec("bo_sb", b_d["o"])
            gs_sb = load_cvec("gs_sb", gs_d)
            gb_sb = load_cvec("gb_sb", gb_d)

            bv_bc = cpool.tile([128, C], F32, tag="ot_sb")  # bv broadcast
            nc.sync.dma_start(
                out=bv_bc,
                in_=bass.AP(
                    tensor=b_d["v"].tensor,
                    offset=b_d["v"].offset,
                    ap=[[0, 128]] + list(b_d["v"].ap),
                ),
            )

            # ---- weights: load row-blocks + PE-transpose (plain fp32) into
            #      wT[c, o], converting to fp32r in the PSUM->SBUF copy.
            #      Emitted first: gives the PE work while groupnorm streams x.
            wts = {}
            for nm in ("q", "k", "v", "o"):
                wts[nm] = wt.tile([128, CT, C], F32R, name=f"w{nm}t")
            # rotate transpose psums over the (currently idle) attention tags
            # and alternate the copy-back engine, so ~6 transposes pipeline
            pt_tags = [("s4", 3), ("den", 2), ("av", 2), ("s4", 3),
                       ("den", 2), ("av", 2), ("op", 1)]
            ti = 0
            for nm in ("v", "q", "k", "o"):
                for ot in range(CT):
                    stg = epool.tile([128, C], F32, tag="e2", name="stg")
                    nc.sync.dma_start(
                        out=stg, in_=w_d[nm][ot * 128 : (ot + 1) * 128, :]
                    )
                    for ct in range(CT):
                        tag, tb = pt_tags[ti % len(pt_tags)]
                        pt = psum.tile([128, 128], F32, tag=tag, bufs=tb, name="pt")
                        nc.tensor.transpose(
                            pt, stg[:, ct * 128 : (ct + 1) * 128], identity
                        )
                        dstw = wts[nm][:, ct, ot * 128 : (ot + 1) * 128]
                        if ti % 2 == 0:
                            nc.scalar.copy(dstw, pt)
                        else:
                            nc.vector.tensor_copy(dstw, pt)
                        ti += 1

            # ---- groupnorm stats: stream x chunks ----
            x_r = x_d.rearrange("(t p) l -> p t l", p=128)
            h_sb = big.tile([128, CT, L], F32R, tag="xattn")
            gn_ab = []  # (a_t, b_t) per channel tile
            for ct in range(CT):
                stats = small.tile([128, 4, 6], F32, tag="stats")
                for i in range(4):
                    xc = cpool.tile([128, 512], F32, tag="xc", bufs=4)
                    nc.sync.dma_start(
                        out=xc, in_=x_r[:, ct, i * 512 : (i + 1) * 512]
                    )
                    nc.vector.bn_stats(out=stats[:, i, :], in_=xc)
                mv = small.tile([128, 2], F32, tag="mv")
                nc.vector.bn_aggr(out=mv, in_=stats)
                # stat2 = [mean_p, E[x^2]_p]
                stat2 = small.tile([128, 2], F32, tag="stat2")
                nc.vector.tensor_copy(stat2[:, 0:1], mv[:, 0:1])
                nc.vector.scalar_tensor_tensor(
                    out=stat2[:, 1:2],
                    in0=mv[:, 0:1],
                    scalar=mv[:, 0:1],
                    in1=mv[:, 1:2],
                    op0=ALU.mult,
                    op1=ALU.add,
                )
                pg = psum.tile([128, 2], F32, tag="den")
                nc.tensor.matmul(pg, ones, stat2, start=True, stop=True)
                mean_t = small.tile([128, 1], F32, tag="mean_t")
                nc.vector.tensor_scalar_mul(mean_t, pg[:, 0:1], 1.0 / 128.0)
                ex2_t = small.tile([128, 1], F32, tag="ex2_t")
                nc.vector.tensor_scalar_mul(ex2_t, pg[:, 1:2], 1.0 / 128.0)
                var_t = small.tile([128, 1], F32, tag="var_t")
                nc.vector.tensor_mul(var_t, mean_t, mean_t)
                nc.vector.tensor_sub(var_t, ex2_t, var_t)
                std_t = small.tile([128, 1], F32, tag="std_t")
                nc.scalar.activation(std_t, var_t, AFT.Sqrt, bias=eps_t)
                rstd_t = small.tile([128, 1], F32, tag="rstd_t")
                nc.vector.reciprocal(rstd_t, std_t)
                a_t = small.tile([128, 1], F32, tag="a_t", bufs=CT)
                nc.vector.tensor_mul(a_t, rstd_t, gs_sb[:, ct : ct + 1])
                b_t = small.tile([128, 1], F32, tag="b_t", bufs=CT)
                nc.vector.tensor_mul(b_t, mean_t, a_t)
                nc.vector.tensor_sub(b_t, gb_sb[:, ct : ct + 1], b_t)
                gn_ab.append((a_t, b_t))

            # ---- groupnorm apply: h = a*x + b, written as fp32r.
            #      l-chunk outer so early l-chunks of h complete first. ----
            for lc in range(LC):
                for ct in range(CT):
                    a_t, b_t = gn_ab[ct]
                    xc = cpool.tile([128, 512], F32, tag="xc", bufs=4)
                    nc.sync.dma_start(
                        out=xc, in_=x_r[:, ct, lc * 512 : (lc + 1) * 512]
                    )
                    nc.scalar.activation(
                        h_sb[:, ct, lc * 512 : (lc + 1) * 512],
                        xc,
                        AFT.Identity,
                        bias=b_t,
                        scale=a_t,
                    )

            # ---- vT projection first (attention needs all of it) ----
            vT_sb = big.tile([128, LT, C], F32R, tag="vT_sb")
            for lt in range(LT):
                pp = psum.tile([128, 512], F32, tag="den")
                for ct in range(CT):
                    nc.tensor.matmul(
                        pp,
                        h_sb[:, ct, lt * 128 : (lt + 1) * 128],
                        wts["v"][:, ct, :],
                        start=(ct == 0),
                        stop=(ct == CT - 1),
                    )
                nc.vector.tensor_add(vT_sb[:, lt, :], pp, bv_bc)

            # ---- q, k projections: [d, l], head-major; bias-add + fp32r
            #      conversion on the scalar engine ----
            q_sb = big.tile([128, NH, L], F32R, tag="q_sb")
            k_sb = big.tile([128, NH, L], F32R, tag="k_sb")
            for h in range(NH):
                for dst, wtt, bias in (
                    (q_sb, wts["q"], bq_sb),
                    (k_sb, wts["k"], bk_sb),
                ):
                    for lc in range(LC):
                        pp = psum.tile([128, 512], F32, tag="av")
                        for ct in range(CT):
                            nc.tensor.matmul(
                                pp,
                                wtt[:, ct, h * 128 : (h + 1) * 128],
                                h_sb[:, ct, lc * 512 : (lc + 1) * 512],
                                start=(ct == 0),
                                stop=(ct == CT - 1),
                            )
                        nc.scalar.activation(
                            dst[:, h, lc * 512 : (lc + 1) * 512],
                            pp,
                            AFT.Identity,
                            bias=bias[:, h : h + 1],
                        )

            # ---- attention (q-chunk outer), software-pipelined per k-tile;
            #      out-projection per l-chunk overlaps the next q-chunk ----
            attn_sb = big.tile([128, NH, L], F32R, tag="xattn")

            def emit_qk_exp(h, qc, kt):
                ps = psum.tile([128, 512], F32, tag="s4", bufs=3)
                nc.tensor.matmul(
                    ps,
                    k_sb[:, h, kt * 128 : (kt + 1) * 128],
                    q_sb[:, h, qc * 512 : (qc + 1) * 512],
                    start=True,
                    stop=True,
                )
                e2 = epool.tile([128, 512], F32R, tag="e2")
                nc.scalar.activation(e2, ps, AFT.Exp, scale=SM_SCALE)
                return e2

            KGN = LT // 2  # k-tile pairs

            def emit_den_av(h, qc, kg, e_a, e_b, p8, pden, pav):
                for j, e in ((0, e_a), (1, e_b)):
                    kt = kg * 2 + j
                    nc.tensor.matmul(
                        pav,
                        vT_sb[:, kt, h * 128 : (h + 1) * 128],
                        e,
                        start=(kt == 0),
                        stop=(kt == LT - 1),
                    )
                nc.tensor.matmul(
                    pden, ones_r, p8, start=(kg == 0), stop=(kg == KGN - 1)
                )

            def finish_chunk(h, qc, pden, pav):
                rden = cpool.tile([128, 512], F32, tag="rden", bufs=1, name="rden")
                nc.vector.reciprocal(rden, pden)
                nc.vector.tensor_mul(
                    attn_sb[:, h, qc * 512 : (qc + 1) * 512], pav, rden
                )

            def emit_out_proj(lc, last):
                for ot in range(CT):
                    xr = cpool.tile([128, 512], F32, tag="xc", bufs=4, name="xr")
                    nc.sync.dma_start(
                        out=xr,
                        in_=x_d[
                            ot * 128 : (ot + 1) * 128, lc * 512 : (lc + 1) * 512
                        ],
                    )
                    # the final l-chunk may use the attention "den" slots
                    # (attention is over by then) for 2-deep overlap
                    pp = (
                        psum.tile([128, 512], F32, tag="den", name="pp")
                        if last
                        else psum.tile([128, 512], F32, tag="op", bufs=1, name="pp")
                    )
                    for ct in range(CT):
                        nc.tensor.matmul(
                            pp,
                            wts["o"][:, ct, ot * 128 : (ot + 1) * 128],
                            attn_sb[:, ct, lc * 512 : (lc + 1) * 512],
                            start=(ct == 0),
                            stop=(ct == CT - 1),
                        )
                    ot_sb = cpool.tile([128, 512], F32, tag="ot_sb")
                    nc.vector.scalar_tensor_tensor(
                        out=ot_sb,
                        in0=pp,
                        scalar=bo_sb[:, ot : ot + 1],
                        in1=xr,
                        op0=ALU.add,
                        op1=ALU.add,
                    )
                    nc.sync.dma_start(
                        out=out_d[
                            ot * 128 : (ot + 1) * 128, lc * 512 : (lc + 1) * 512
                        ],
                        in_=ot_sb,
                    )

            def drain_one(pq):
                p = pq.pop(0)
                emit_den_av(*p)
                if p[2] == KGN - 1:
                    finish_chunk(p[0], p[1], p[6], p[7])

            deferred_out = None  # l-chunk whose out-projection awaits emission
            for qc in range(LC):
                pipeline = []
                for h in range(NH):
                    pden = psum.tile([128, 512], F32, tag="den")
                    pav = psum.tile([128, 512], F32, tag="av")
                    for kg in range(KGN):
                        e_a = emit_qk_exp(h, qc, kg * 2)
                        e_b = emit_qk_exp(h, qc, kg * 2 + 1)
                        p8 = epool.tile([128, 512], F32R, tag="p8", bufs=2)
                        nc.vector.tensor_add(p8, e_a, e_b)
                        if pipeline:
                            drain_one(pipeline)
                        pipeline.append((h, qc, kg, e_a, e_b, p8, pden, pav))
                        # emit the previous q-chunk's out-projection a few
                        # k-tiles into this one, so the PE queue has ready
                        # attention work while that chain completes
                        if deferred_out is not None and h == 0 and kg == 1:
                            emit_out_proj(deferred_out, last=False)
                            deferred_out = None
                # flush so the out-projection sees completed attention columns
                while pipeline:
                    drain_one(pipeline)
                deferred_out = qc
            emit_out_proj(deferred_out, last=True)
    nc.compile()
    return nc


_NC_CACHE = {}


def _get_nc():
    if "nc" not in _NC_CACHE:
        nc = bacc.Bacc("TRN2", debug=False)
        build_attn_block(nc)
        _NC_CACHE["nc"] = nc
    return _NC_CACHE["nc"]


def run(trace=False, **inputs):
    nc = _get_nc()
    xs = np.ascontiguousarray(np.asarray(inputs["x"], dtype=np.float32))
    shared = {}
    for nm in ("gn_scale", "gn_bias", "wq", "bq", "wk", "bk", "wv", "bv", "wo", "bo"):
        shared[nm] = np.ascontiguousarray(np.asarray(inputs[nm], dtype=np.float32))
    in_maps = [dict(shared, x=xs[b]) for b in range(B)]
    res = run_bass_kernel_spmd(nc, in_maps, core_ids=list(range(B)), trace=trace)
    out = np.stack([res.results[b]["out"] for b in range(B)], axis=0)
    return out, res


def kernel(**inputs):
    out, _ = run(trace=bool(os.environ.get("ATTN_TRACE")), **inputs)
    return out
